# revision 1
# baseline (speedup 1.0000x reference)
"""Trainium2 Bass kernel for nn_Net_23210003267823 (BiGCN rumor-detection net).

Math (per branch, edge set A, weights W1,b1,W2,b2):
    U  = x @ W1                                  (big GEMM, memory-bound: x is 400 MB)
    Y  = D^-1/2 U ;  h1 = D^-1/2 (A Y + Y) + b1  (sym-normalized GCN conv w/ self loops)
    Q  = relu(x[root]) @ W2[64:]                 (root-extend folded: only 128 distinct root rows)
    z  = relu(h1) @ W2[:64] + Q[batch]
    h2 = relu(D^-1/2 (A Zt + Zt) + b2),  Zt = D^-1/2 z
    out_branch = [segment_mean(h2, batch) | h1[root] * (cnt>0)]
Final: log_softmax(concat(td, bu) @ fc_W + fc_b).

Sharding: nodes row-sharded over 8 cores (2500 real + 60 pad rows each).
AllGather of per-branch 64-wide f32 message tables; aggregation via one
dma_gather per (dst-block, branch) + is_equal one-hot matmuls into PSUM.
Host prep is integer index metadata only (edge partition/sort, degree counts).
"""
import sys, os
sys.path.insert(0, "/opt/trn_rl_repo")
import numpy as np

NC_ = 8
N, E, G = 20000, 320000, 128
IN, HID, OUT = 5000, 64, 64
RPC, PRC, NBLK = 2500, 2560, 20   # real rows/core, padded rows/core, row blocks
NPAD = NC_ * PRC                   # 20480
INP, NK = 5120, 40                 # padded IN, K blocks
BIG = np.float32(1e30)

_cache = {}


def _wrap16(idx):
    """dma_gather wrapped-index layout: [128, n/16] i16, idx i at (p = i%16 (replicated), c = i//16)."""
    n = idx.shape[-1]
    out = np.zeros(idx.shape[:-1] + (128, n // 16), np.int16)
    cols = np.arange(n // 16)
    for p in range(128):
        out[..., p, :] = idx[..., cols * 16 + (p % 16)]
    return out


def _build(TB):
    KSTOP = int(os.environ.get("KSTOP", "99"))
    import concourse.bass as bass
    import concourse.mybir as mybir
    import concourse.tile as tile
    from concourse import bacc, library_config

    dt = mybir.dt
    f32, bf16, i32, i16 = dt.float32, dt.bfloat16, dt.int32, dt.int16
    AF = mybir.ActivationFunctionType
    OP = mybir.AluOpType

    nc = bacc.Bacc("TRN2", target_bir_lowering=False, debug=False, num_devices=NC_)

    # ---------------- I/O ----------------
    xc = nc.dram_tensor("xc", [RPC, IN], f32, kind="ExternalInput")
    w1 = nc.dram_tensor("w1", [IN, 128], f32, kind="ExternalInput")
    w2a = nc.dram_tensor("w2a", [128, 128], f32, kind="ExternalInput")
    w2b = nc.dram_tensor("w2b", [IN, 128], f32, kind="ExternalInput")
    bias1 = nc.dram_tensor("bias1", [128, 128], f32, kind="ExternalInput")
    bias2 = nc.dram_tensor("bias2", [128, 128], f32, kind="ExternalInput")
    deg = nc.dram_tensor("deg", [2, PRC], f32, kind="ExternalInput")
    srcs = nc.dram_tensor("srcs", [2, NBLK, 128, TB * 8], i16, kind="ExternalInput")
    drel = nc.dram_tensor("drel", [2, NBLK, 128, TB], f32, kind="ExternalInput")
    brel = nc.dram_tensor("brel", [PRC], f32, kind="ExternalInput")
    bidx = nc.dram_tensor("bidx", [128, PRC // 16], i16, kind="ExternalInput")
    rloc = nc.dram_tensor("rloc", [G], i32, kind="ExternalInput")
    rxloc = nc.dram_tensor("rxloc", [G], i32, kind="ExternalInput")
    iota_in = nc.dram_tensor("iota_in", [128, 128], f32, kind="ExternalInput")
    fcw = nc.dram_tensor("fcw", [2, 128, 256], f32, kind="ExternalInput")
    fcb = nc.dram_tensor("fcb", [128, 2], f32, kind="ExternalInput")
    out = nc.dram_tensor("out", [G, 2], f32, kind="ExternalOutput")
    DBG = os.environ.get("KDBG", "0") == "1"
    if DBG:
        dbgY = nc.dram_tensor("dbgY", [PRC, 128], f32, kind="ExternalOutput")
        dbgZ = nc.dram_tensor("dbgZ", [PRC, 128], f32, kind="ExternalOutput")
        dbgH = nc.dram_tensor("dbgH", [PRC, 128], f32, kind="ExternalOutput")
        dbgQ = nc.dram_tensor("dbgQ", [G, 128], f32, kind="ExternalOutput")

    # ---------------- internal DRAM ----------------
    Ytl = nc.dram_tensor("Ytl", [PRC, 128], bf16)
    Ytf = nc.dram_tensor("Ytf", [NPAD, 128], bf16, addr_space="Shared")
    Ztl = nc.dram_tensor("Ztl", [PRC, 128], bf16)
    Ztf = nc.dram_tensor("Ztf", [NPAD, 128], bf16, addr_space="Shared")
    h1loc = nc.dram_tensor("h1loc", [PRC + 1, 128], f32)
    Qtab = nc.dram_tensor("Qtab", [G + 1, 128], f32, addr_space="Shared")
    qbl = nc.dram_tensor("qbl", [G, 128], f32)
    arl = nc.dram_tensor("arl", [128, 257], f32)
    arf = nc.dram_tensor("arf", [128, 257], f32, addr_space="Shared")

    RG = [list(range(NC_))]
    NE = TB * 128  # edges (padded) per (blk, br)

    with tile.TileContext(nc) as tc:
        with tc.tile_pool(name="const", bufs=1) as cp:
            nc.gpsimd.load_library(library_config.mlp)

            iof = cp.tile([128, 128], f32)
            nc.sync.dma_start(out=iof[:], in_=iota_in[:])

            # dinv [128, 40]: col br*NBLK+blk
            dga = cp.tile([128, NBLK * 2], f32)
            nc.sync.dma_start(out=dga[:], in_=deg[:].rearrange("t (b p) -> p (t b)", p=128))
            drc = cp.tile([128, NBLK * 2], f32)
            nc.vector.reciprocal(drc[:], dga[:])
            dinv = cp.tile([128, NBLK * 2], f32)
            nc.scalar.activation(dinv[:], drc[:], AF.Sqrt)

            b1t = cp.tile([128, 128], f32)
            nc.sync.dma_start(out=b1t[:], in_=bias1[:])
            b2t = cp.tile([128, 128], f32)
            nc.sync.dma_start(out=b2t[:], in_=bias2[:])
            w2at = cp.tile([128, 128], bf16)
            nc.gpsimd.dma_start(out=w2at[:], in_=w2a[:])
            brelt = cp.tile([128, NBLK], f32)
            nc.sync.dma_start(out=brelt[:], in_=brel[:].rearrange("(b p) -> p b", p=128))
            bidxt = cp.tile([128, PRC // 16], i16)
            nc.sync.dma_start(out=bidxt[:], in_=bidx[:])
            rloct = cp.tile([128, 1], i32)
            nc.sync.dma_start(out=rloct[:], in_=rloc[:, None])
            rxloct = cp.tile([128, 1], i32)
            nc.sync.dma_start(out=rxloct[:], in_=rxloc[:, None])
            fcw0 = cp.tile([128, 256], f32)
            nc.sync.dma_start(out=fcw0[:], in_=fcw[0])
            fcw1 = cp.tile([128, 256], f32)
            nc.sync.dma_start(out=fcw1[:], in_=fcw[1])
            fcbt = cp.tile([128, 2], f32)
            nc.sync.dma_start(out=fcbt[:], in_=fcb[:])

            # zero rows for h1loc[2560] and Qtab[128]
            zrow = cp.tile([1, 128], f32)
            nc.vector.memset(zrow[:], 0.0)
            nc.sync.dma_start(out=h1loc[PRC:PRC + 1, :], in_=zrow[:])
            nc.sync.dma_start(out=Qtab[G:G + 1, :], in_=zrow[:])

            # ---------------- phase R: root rows -> Q (partial) ----------------
            if KSTOP >= 1:
             with tc.tile_pool(name="pr", bufs=2) as pr, \
                 tc.tile_pool(name="prp", bufs=1, space="PSUM") as prp:
                Rt_ = pr.tile([128, INP], f32, tag="rbig")
                nc.vector.memset(Rt_[:], 0.0)
                nc.gpsimd.indirect_dma_start(
                    out=Rt_[:, 0:IN], out_offset=None, in_=xc[:],
                    in_offset=bass.IndirectOffsetOnAxis(ap=rxloct[:, :1], axis=0),
                    bounds_check=RPC - 1, oob_is_err=False)
                Rr = pr.tile([128, INP], bf16, tag="rbig2")
                nc.scalar.activation(Rr[:], Rt_[:], AF.Relu)
                w2ball = pr.tile([128, NK * 128], bf16, tag="w2ball")
                nc.vector.memset(w2ball[:, 39 * 128:], 0.0)
                nc.gpsimd.dma_start(out=w2ball[:, 0:39 * 128].rearrange("p (k f) -> p k f", f=128),
                                    in_=w2b[0:4992, :].rearrange("(k p) f -> p k f", p=128))
                nc.gpsimd.dma_start(out=w2ball[0:8, 39 * 128:40 * 128], in_=w2b[4992:IN, :])
                pq = prp.tile([128, 128], f32)
                rtall = pr.tile([128, NK, 128], bf16, tag="rtall")
                nc.sync.dma_start(out=rtall[:], in_=Rr[:], transpose=True)
                for k in range(NK):
                    nc.tensor.matmul(out=pq[:], lhsT=rtall[:, k, :], rhs=w2ball[:, k * 128:(k + 1) * 128],
                                     start=(k == 0), stop=(k == NK - 1))
                qsb = pr.tile([128, 128], f32, tag="qsb")
                nc.vector.tensor_copy(qsb[:], pq[:])
                nc.sync.dma_start(out=qbl[:], in_=qsb[:])
            if KSTOP >= 1:
             nc.gpsimd.collective_compute("AllReduce", OP.add, replica_groups=RG,
                                          ins=[qbl[:]], outs=[Qtab[0:G, :]])

            # ---------------- phase G: U^T = W1^T x^T ; Y ----------------
            if KSTOP >= 2:
             with tc.tile_pool(name="pw", bufs=1) as pw, \
                 tc.tile_pool(name="px", bufs=5) as px, \
                 tc.tile_pool(name="pxt", bufs=3) as pxt, \
                 tc.tile_pool(name="pub", bufs=3) as pub, \
                 tc.tile_pool(name="pup", bufs=2, space="PSUM") as pup:
                w1all = pw.tile([128, NK * 128], bf16)
                nc.vector.memset(w1all[:, 39 * 128:], 0.0)
                nc.gpsimd.dma_start(out=w1all[:, 0:39 * 128].rearrange("p (k f) -> p k f", f=128),
                                    in_=w1[0:4992, :].rearrange("(k p) f -> p k f", p=128))
                nc.gpsimd.dma_start(out=w1all[0:8, 39 * 128:40 * 128], in_=w1[4992:IN, :])

                for rc in range(5):
                    xbs = []
                    for j in range(4):
                        bi = rc * 4 + j
                        row0 = bi * 128
                        nr = min(128, RPC - row0)
                        xb = px.tile([128, INP], bf16, tag="xb")
                        if nr < 128:
                            nc.vector.memset(xb[:], 0.0)
                        else:
                            nc.vector.memset(xb[:, IN:INP], 0.0)
                        nc.gpsimd.dma_start(out=xb[0:nr, 0:IN], in_=xc[row0:row0 + nr, :])
                        xbs.append(xb)
                    pu = pup.tile([128, 512], f32)
                    xtc = pxt.tile([128, NK, 4, 128], bf16, tag="xtc")
                    for j in range(4):
                        nc.sync.dma_start(out=xtc[:, :, j, :], in_=xbs[j][:], transpose=True)
                    for k in range(NK):
                        nc.tensor.matmul(out=pu[:], lhsT=w1all[:, k * 128:(k + 1) * 128], rhs=xtc[:, k, :, :],
                                         start=(k == 0), stop=(k == NK - 1))
                    ut = pub.tile([128, 512], bf16, tag="ut")
                    nc.vector.tensor_copy(ut[:], pu[:])
                    ubt = pub.tile([128, 4, 128], bf16, tag="ubt")
                    nc.sync.dma_start(out=ubt[:], in_=ut[:], transpose=True)
                    for j in range(4):
                        bi = rc * 4 + j
                        yb = pub.tile([128, 128], bf16, tag="yb")
                        nc.vector.tensor_scalar(out=yb[:, 0:64], in0=ubt[:, j, 0:64],
                                                scalar1=dinv[:, bi:bi + 1], scalar2=None, op0=OP.mult)
                        nc.vector.tensor_scalar(out=yb[:, 64:128], in0=ubt[:, j, 64:128],
                                                scalar1=dinv[:, NBLK + bi:NBLK + bi + 1], scalar2=None, op0=OP.mult)
                        nc.sync.dma_start(out=Ytl[bi * 128:(bi + 1) * 128, :], in_=yb[:])

            if KSTOP >= 3:
             nc.gpsimd.collective_compute("AllGather", OP.bypass, replica_groups=RG,
                                          ins=[Ytl[:]], outs=[Ytf[:]])

            # ---------------- conv helper ----------------
            def agg_block(pools, table, blk, br):
                """A @ table for dst block blk, branch br -> psum tile [128,64] (f32)"""
                pa, pv, po, ph = pools
                st = pa.tile([128, TB * 8], i16, tag="st")
                nc.sync.dma_start(out=st[:], in_=srcs[br, blk])
                dr_ = pa.tile([128, TB], f32, tag="dr")
                nc.sync.dma_start(out=dr_[:], in_=drel[br, blk])
                V = pv.tile([128, TB, 128], bf16, tag="v")
                nc.gpsimd.dma_gather(V[:], table[:], st[:], NE, NE, 128, single_packet=False)
                oh = po.tile([128, TB, 128], bf16, tag="oh")
                nc.vector.tensor_tensor(out=oh[:],
                                        in0=dr_[:, :, None].to_broadcast([128, TB, 128]),
                                        in1=iof[:, None, :].to_broadcast([128, TB, 128]),
                                        op=OP.is_equal)
                ph_ = ph.tile([128, 64], f32)
                for t in range(TB):
                    nc.tensor.matmul(out=ph_[:], lhsT=oh[:, t, :], rhs=V[:, t, br * 64:(br + 1) * 64],
                                     start=(t == 0), stop=(t == TB - 1))
                return ph_

            # ---------------- phase C1: conv1 -> h1, z, Zt ----------------
            if KSTOP >= 4:
             with tc.tile_pool(name="pa1", bufs=5) as pa, \
                 tc.tile_pool(name="pv1", bufs=5) as pv, \
                 tc.tile_pool(name="po1", bufs=5) as po, \
                 tc.tile_pool(name="pm1", bufs=3) as pm, \
                 tc.tile_pool(name="pq1", bufs=1) as pq1, \
                 tc.tile_pool(name="ph1", bufs=3, space="PSUM") as ph, \
                 tc.tile_pool(name="pz1", bufs=2, space="PSUM") as pz:
                pools = (pa, pv, po, ph)
                # one merged gather of Q[batch] for all rows
                qall = pq1.tile([128, NBLK, 128], f32)
                nc.gpsimd.dma_gather(qall[:], Qtab[:], bidxt[:], PRC, PRC, 128, single_packet=False)
                for blk in range(NBLK):
                    h1f = pm.tile([128, 128], f32, tag="h1f")
                    h1b = pm.tile([128, 128], bf16, tag="h1b")
                    for br in range(2):
                        ph_ = agg_block(pools, Ytf, blk, br)
                        ys = pm.tile([128, 64], bf16, tag="ys")
                        nc.sync.dma_start(out=ys[:], in_=Ytl[blk * 128:(blk + 1) * 128, br * 64:(br + 1) * 64])
                        hs = pm.tile([128, 64], f32, tag="hs")
                        nc.vector.tensor_tensor(out=hs[:], in0=ph_[:], in1=ys[:], op=OP.add)
                        nc.vector.tensor_scalar(out=hs[:], in0=hs[:],
                                                scalar1=dinv[:, br * NBLK + blk:br * NBLK + blk + 1],
                                                scalar2=None, op0=OP.mult)
                        nc.vector.tensor_tensor(out=h1f[:, br * 64:(br + 1) * 64], in0=hs[:],
                                                in1=b1t[:, br * 64:(br + 1) * 64], op=OP.add)
                        nc.vector.tensor_tensor(out=h1b[:, br * 64:(br + 1) * 64], in0=hs[:],
                                                in1=b1t[:, br * 64:(br + 1) * 64], op=OP.add)
                    nc.sync.dma_start(out=h1loc[blk * 128:(blk + 1) * 128, :], in_=h1f[:])
                    hr = pm.tile([128, 128], bf16, tag="hr")
                    nc.scalar.activation(hr[:], h1b[:], AF.Relu)
                    hrT = pm.tile([128, 128], bf16, tag="hrT")
                    nc.sync.dma_start(out=hrT[:], in_=hr[:], transpose=True)
                    pz_ = pz.tile([128, 128], f32)
                    nc.tensor.matmul(out=pz_[:], lhsT=hrT[:], rhs=w2at[:], start=True, stop=True)
                    zf = pm.tile([128, 128], f32, tag="zf")
                    nc.vector.tensor_tensor(out=zf[:], in0=pz_[:], in1=qall[:, blk, :], op=OP.add)
                    ztb = pm.tile([128, 128], bf16, tag="ztb")
                    nc.vector.tensor_scalar(out=ztb[:, 0:64], in0=zf[:, 0:64],
                                            scalar1=dinv[:, blk:blk + 1], scalar2=None, op0=OP.mult)
                    nc.vector.tensor_scalar(out=ztb[:, 64:128], in0=zf[:, 64:128],
                                            scalar1=dinv[:, NBLK + blk:NBLK + blk + 1], scalar2=None, op0=OP.mult)
                    nc.sync.dma_start(out=Ztl[blk * 128:(blk + 1) * 128, :], in_=ztb[:])

            if DBG and KSTOP >= 4:
             with tc.tile_pool(name="pdbg", bufs=2) as pd:
                for b in range(NBLK):
                    t1 = pd.tile([128, 128], f32, tag="t1")
                    nc.gpsimd.dma_start(out=t1[:], in_=Ytl[b * 128:(b + 1) * 128, :])
                    nc.sync.dma_start(out=dbgY[b * 128:(b + 1) * 128, :], in_=t1[:])
                    t2 = pd.tile([128, 128], f32, tag="t2")
                    nc.gpsimd.dma_start(out=t2[:], in_=Ztl[b * 128:(b + 1) * 128, :])
                    nc.sync.dma_start(out=dbgZ[b * 128:(b + 1) * 128, :], in_=t2[:])
                    t3 = pd.tile([128, 128], f32, tag="t3")
                    nc.sync.dma_start(out=t3[:], in_=h1loc[b * 128:(b + 1) * 128, :])
                    nc.sync.dma_start(out=dbgH[b * 128:(b + 1) * 128, :], in_=t3[:])
                tq = pd.tile([128, 128], f32, tag="tq")
                nc.sync.dma_start(out=tq[:], in_=Qtab[0:G, :])
                nc.sync.dma_start(out=dbgQ[:], in_=tq[:])
            if KSTOP >= 5:
             nc.gpsimd.collective_compute("AllGather", OP.bypass, replica_groups=RG,
                                          ins=[Ztl[:]], outs=[Ztf[:]])

            # ---------------- phase C2: conv2 -> h2 -> segment sums ----------------
            if KSTOP >= 6:
             with tc.tile_pool(name="pa2", bufs=5) as pa2, \
                 tc.tile_pool(name="pv2", bufs=5) as pv2, \
                 tc.tile_pool(name="po2", bufs=5) as po2, \
                 tc.tile_pool(name="pm2", bufs=3) as pm2, \
                 tc.tile_pool(name="ph2", bufs=3, space="PSUM") as ph2, \
                 tc.tile_pool(name="ps2", bufs=1, space="PSUM") as ps2:
                pools2 = (pa2, pv2, po2, ph2)
                pseg = ps2.tile([128, 129], f32)
                for blk in range(NBLK):
                    pay = pm2.tile([128, 129], f32, tag="pay")
                    nc.vector.memset(pay[:, 128:129], 1.0)
                    for br in range(2):
                        ph_ = agg_block(pools2, Ztf, blk, br)
                        zs = pm2.tile([128, 64], bf16, tag="zs")
                        nc.sync.dma_start(out=zs[:], in_=Ztl[blk * 128:(blk + 1) * 128, br * 64:(br + 1) * 64])
                        hs2 = pm2.tile([128, 64], f32, tag="hs2")
                        nc.vector.tensor_tensor(out=hs2[:], in0=ph_[:], in1=zs[:], op=OP.add)
                        nc.vector.tensor_scalar(out=hs2[:], in0=hs2[:],
                                                scalar1=dinv[:, br * NBLK + blk:br * NBLK + blk + 1],
                                                scalar2=None, op0=OP.mult)
                        nc.vector.tensor_tensor(out=hs2[:], in0=hs2[:],
                                                in1=b2t[:, br * 64:(br + 1) * 64], op=OP.add)
                        nc.scalar.activation(pay[:, br * 64:(br + 1) * 64], hs2[:], AF.Relu)
                    ohs = pm2.tile([128, 128], f32, tag="ohs")
                    nc.vector.tensor_tensor(out=ohs[:], in0=brelt[:, blk:blk + 1].to_broadcast([128, 128]),
                                            in1=iof[:], op=OP.is_equal)
                    nc.tensor.matmul(out=pseg[:], lhsT=ohs[:], rhs=pay[:], start=(blk == 0), stop=(blk == NBLK - 1))

                rg = pm2.tile([128, 128], f32, tag="rg")
                nc.gpsimd.indirect_dma_start(
                    out=rg[:], out_offset=None, in_=h1loc[:],
                    in_offset=bass.IndirectOffsetOnAxis(ap=rloct[:, :1], axis=0))
                part = pm2.tile([128, 257], f32, tag="part")
                nc.vector.tensor_copy(part[:, 0:129], pseg[:])
                nc.vector.tensor_copy(part[:, 129:257], rg[:])
                nc.sync.dma_start(out=arl[:], in_=part[:])

            if KSTOP >= 7:
             nc.gpsimd.collective_compute("AllReduce", OP.add, replica_groups=RG,
                                          ins=[arl[:]], outs=[arf[:]])

            # ---------------- final ----------------
            if KSTOP >= 7:
             with tc.tile_pool(name="pf", bufs=1) as pf:
                Rt = pf.tile([128, 257], f32)
                nc.sync.dma_start(out=Rt[:], in_=arf[:])
                cnt = Rt[:, 128:129]
                c1 = pf.tile([128, 1], f32)
                nc.vector.tensor_scalar_max(out=c1[:], in0=cnt, scalar1=1.0)
                rec = pf.tile([128, 1], f32)
                nc.vector.reciprocal(rec[:], c1[:])
                ind = pf.tile([128, 1], f32)
                nc.vector.tensor_scalar_min(out=ind[:], in0=cnt, scalar1=1.0)
                hfc = pf.tile([128, 256], f32)
                nc.vector.tensor_scalar(out=hfc[:, 0:64], in0=Rt[:, 0:64], scalar1=rec[:, :1], scalar2=None, op0=OP.mult)
                nc.vector.tensor_scalar(out=hfc[:, 64:128], in0=Rt[:, 129:193], scalar1=ind[:, :1], scalar2=None, op0=OP.mult)
                nc.vector.tensor_scalar(out=hfc[:, 128:192], in0=Rt[:, 64:128], scalar1=rec[:, :1], scalar2=None, op0=OP.mult)
                nc.vector.tensor_scalar(out=hfc[:, 192:256], in0=Rt[:, 193:257], scalar1=ind[:, :1], scalar2=None, op0=OP.mult)
                lg = pf.tile([128, 2], f32)
                for j, fw in enumerate((fcw0, fcw1)):
                    tmp = pf.tile([128, 256], f32, tag=f"tmp{j}")
                    nc.vector.tensor_tensor(out=tmp[:], in0=hfc[:], in1=fw[:], op=OP.mult)
                    nc.vector.reduce_sum(lg[:, j:j + 1], tmp[:], axis=mybir.AxisListType.X)
                nc.vector.tensor_tensor(out=lg[:], in0=lg[:], in1=fcbt[:], op=OP.add)
                mx = pf.tile([128, 1], f32)
                nc.vector.reduce_max(mx[:], lg[:], axis=mybir.AxisListType.X)
                d_ = pf.tile([128, 2], f32)
                nc.vector.tensor_scalar(out=d_[:], in0=lg[:], scalar1=mx[:, :1], scalar2=None, op0=OP.subtract)
                e_ = pf.tile([128, 2], f32)
                nc.scalar.activation(e_[:], d_[:], AF.Exp)
                s_ = pf.tile([128, 1], f32)
                nc.vector.reduce_sum(s_[:], e_[:], axis=mybir.AxisListType.X)
                ls = pf.tile([128, 1], f32)
                nc.scalar.activation(ls[:], s_[:], AF.Ln)
                ov = pf.tile([128, 2], f32)
                nc.vector.tensor_scalar(out=ov[:], in0=d_[:], scalar1=ls[:, :1], scalar2=None, op0=OP.subtract)
                nc.sync.dma_start(out=out[:], in_=ov[:])

    nc.compile()
    return nc


def _prep(x, edge_index, bu_edge_index, batch, root_index,
          W1_td, b1_td, W2_td, b2_td, W1_bu, b1_bu, W2_bu, b2_bu, fc_W, fc_b):
    """Host-side: integer index metadata + parameter reshaping (no float math on data)."""
    x = np.asarray(x, np.float32)
    batch = np.asarray(batch).astype(np.int64)
    root_index = np.asarray(root_index).astype(np.int64)
    edges = [np.asarray(edge_index).astype(np.int64), np.asarray(bu_edge_index).astype(np.int64)]

    degs = []
    for ei in edges:
        d = np.bincount(ei[1], minlength=N).astype(np.int64) + 1
        degs.append(d)

    maxcnt = 0
    blk_edges = [[[None] * NBLK for _ in range(2)] for _ in range(NC_)]
    for br, ei in enumerate(edges):
        src, dst = ei[0], ei[1]
        c = dst // RPC
        loc = dst - c * RPC
        blk = loc // 128
        rel = loc - blk * 128
        ps = (src // RPC) * PRC + (src - (src // RPC) * RPC)
        key = c * NBLK + blk
        order = np.argsort(key, kind="stable")
        ks = key[order]
        bounds = np.searchsorted(ks, np.arange(NC_ * NBLK + 1))
        for c_ in range(NC_):
            for b_ in range(NBLK):
                sl = order[bounds[c_ * NBLK + b_]:bounds[c_ * NBLK + b_ + 1]]
                blk_edges[c_][br][b_] = (ps[sl], rel[sl])
                maxcnt = max(maxcnt, len(sl))
    TB = max(1, (maxcnt + 127) // 128)

    srcs_flat = np.zeros((NC_, 2, NBLK, TB * 128), np.int64)
    drel = np.full((NC_, 2, NBLK, 128, TB), -1.0, np.float32)
    for c in range(NC_):
        for br in range(2):
            for b in range(NBLK):
                s, r = blk_edges[c][br][b]
                n = len(s)
                srcs_flat[c, br, b, :n] = s
                lane, til = np.arange(n) % 128, np.arange(n) // 128
                drel[c, br, b, lane, til] = r
    srcs16 = _wrap16(srcs_flat.reshape(NC_ * 2 * NBLK, TB * 128)).reshape(NC_, 2, NBLK, 128, TB * 8)

    deg = np.full((NC_, 2, PRC), BIG, np.float32)
    for br in range(2):
        deg[:, br, :RPC] = degs[br].reshape(NC_, RPC).astype(np.float32)

    brel = np.full((NC_, PRC), -1.0, np.float32)
    brel[:, :RPC] = batch.reshape(NC_, RPC).astype(np.float32)
    bidx_flat = np.full((NC_, PRC), G, np.int64)
    bidx_flat[:, :RPC] = batch.reshape(NC_, RPC)
    bidx16 = _wrap16(bidx_flat)  # [NC_, 128, PRC//16]

    rc = root_index // RPC
    rl = root_index - rc * RPC
    rloc = np.full((NC_, G), PRC, np.int32)
    rxloc = np.full((NC_, G), 1 << 20, np.int32)
    for g in range(G):
        rloc[rc[g], g] = rl[g]
        rxloc[rc[g], g] = rl[g]

    # parameters (pure reshapes / replication)
    w1 = np.hstack([np.asarray(W1_td, np.float32), np.asarray(W1_bu, np.float32)])        # [5000,128]
    w2a = np.zeros((128, 128), np.float32)  # block-diag: one K=128 matmul covers both branches
    w2a[0:64, 0:64] = np.asarray(W2_td, np.float32)[:HID]
    w2a[64:128, 64:128] = np.asarray(W2_bu, np.float32)[:HID]
    w2b = np.hstack([np.asarray(W2_td, np.float32)[HID:], np.asarray(W2_bu, np.float32)[HID:]])  # [5000,128]
    bias1 = np.broadcast_to(np.concatenate([np.asarray(b1_td, np.float32), np.asarray(b1_bu, np.float32)]), (128, 128)).copy()
    bias2 = np.broadcast_to(np.concatenate([np.asarray(b2_td, np.float32), np.asarray(b2_bu, np.float32)]), (128, 128)).copy()
    fcw = np.stack([np.broadcast_to(np.asarray(fc_W, np.float32)[:, j], (128, 256)) for j in range(2)])
    fcb = np.broadcast_to(np.asarray(fc_b, np.float32), (128, 2)).copy()
    iota_in = np.tile(np.arange(128, dtype=np.float32), (128, 1))

    in_maps = []
    for c in range(NC_):
        in_maps.append(dict(
            xc=np.ascontiguousarray(x[c * RPC:(c + 1) * RPC]),
            w1=w1, w2a=w2a, w2b=w2b, bias1=bias1, bias2=bias2,
            deg=np.ascontiguousarray(deg[c]),
            srcs=np.ascontiguousarray(srcs16[c]), drel=np.ascontiguousarray(drel[c]),
            brel=np.ascontiguousarray(brel[c]), bidx=np.ascontiguousarray(bidx16[c]),
            rloc=np.ascontiguousarray(rloc[c]), rxloc=np.ascontiguousarray(rxloc[c]),
            iota_in=iota_in, fcw=np.ascontiguousarray(fcw), fcb=fcb,
        ))
    return TB, in_maps


def kernel(**inputs):
    from concourse.bass_utils import run_bass_kernel_spmd
    TB, in_maps = _prep(**inputs)
    if TB not in _cache:
        _cache[TB] = _build(TB)
    nc = _cache[TB]
    res = run_bass_kernel_spmd(nc, in_maps, list(range(NC_)))
    return res.results[0]["out"]


if __name__ == "__main__":
    import reference
    inputs = {k: np.asarray(v) for k, v in reference.setup_inputs().items()}
    got = kernel(**inputs)
    print(got[:4])



# revision 2
# speedup vs baseline: 5.5409x; 5.5409x over previous
"""Trainium2 Bass kernel for nn_Net_23210003267823 (BiGCN rumor-detection net).

Math (per branch, edge set A, weights W1,b1,W2,b2):
    U  = x @ W1                                  (big GEMM, memory-bound)
    Y  = D^-1/2 U ;  h1 = D^-1/2 (A Y + Y) + b1  (sym-normalized GCN conv w/ self loops)
    Q  = relu(x[root]) @ W2[64:]                 (root-extend folded: only 128 distinct root rows)
    z  = relu(h1) @ W2[:64] + Q[batch]
    h2 = relu(D^-1/2 (A Zt + Zt) + b2),  Zt = D^-1/2 z
    out_branch = [segment_mean(h2, batch) | h1[root] * (cnt>0)]
Final: log_softmax(concat(td, bu) @ fc_W + fc_b).

v3 layout: host pre-transposes x to bf16 K-major per core (halves the HBM
read, kills on-device transposes), ships the 128 root rows globally (kills
the Q AllReduce; Q computed during the Y AllGather), one-hots via
tensor_scalar is_equal in bf16 (DVE 2x/4x mode), 5-block grouped gathers
(amortize SWDGE descriptor generation + DMA fixed costs), edge indices
loaded once and shared by both convs, single consolidated stores for the
Y/Z/h1 tables, PE-transpose instead of per-block DMA transposes.

Sharding: nodes row-sharded over 8 cores (2500 real + 60 pad rows each).
AllGather of per-branch 64-wide message tables; aggregation via one
dma_gather per (5-block group, branch) + is_equal one-hot matmuls into PSUM.
Host prep is layout only (transpose/cast/pad, edge partition/sort, degrees).
"""
import sys, os
sys.path.insert(0, "/opt/trn_rl_repo")
import numpy as np
import ml_dtypes

BF16 = ml_dtypes.bfloat16
NC_ = 8
N, E, G = 20000, 320000, 128
IN, HID, OUT = 5000, 64, 64
RPC, PRC, NBLK = 2500, 2560, 20   # real rows/core, padded rows/core, row blocks
GRP, NGRP = 5, 4                   # dst blocks per gather group
NPAD = NC_ * PRC                   # 20480
INP, NK = 5120, 40                 # padded IN, K blocks
BIG = np.float32(1e30)

_cache = {}


def _wrap16(idx):
    """dma_gather wrapped-index layout: [128, n/16] i16, idx i at (p = i%16 (replicated), c = i//16)."""
    n = idx.shape[-1]
    out = np.zeros(idx.shape[:-1] + (128, n // 16), np.int16)
    cols = np.arange(n // 16)
    for p in range(128):
        out[..., p, :] = idx[..., cols * 16 + (p % 16)]
    return out


def _kblocked(w, cols):
    """[IN, cols] f32 -> [128, NK*cols] bf16 with out[p, k*cols+f] = w[k*128+p, f]."""
    wp = np.zeros((INP, cols), np.float32)
    wp[:IN] = w
    return np.ascontiguousarray(
        wp.reshape(NK, 128, cols).transpose(1, 0, 2).reshape(128, NK * cols)).astype(BF16)


def _build(TB):
    KSTOP = int(os.environ.get("KSTOP", "99"))
    import concourse.bass as bass
    import concourse.mybir as mybir
    import concourse.tile as tile
    from concourse import bacc, library_config

    dt = mybir.dt
    f32, bf16, i32, i16 = dt.float32, dt.bfloat16, dt.int32, dt.int16
    AF = mybir.ActivationFunctionType
    OP = mybir.AluOpType

    nc = bacc.Bacc("TRN2", target_bir_lowering=False, debug=False, num_devices=NC_,
                   num_swdge_queues=4)

    GTB = GRP * TB                 # tiles per gather group
    GC = GTB * 8                   # idx cols per (br, grp) segment in st_all

    # ---------------- I/O ----------------
    xth = nc.dram_tensor("xth", [NBLK, (NK // 2) * 128, 256], bf16, kind="ExternalInput")
    xrt = nc.dram_tensor("xrt", [128, NK * 128], bf16, kind="ExternalInput")
    w1p = nc.dram_tensor("w1p", [128, NK * 128], bf16, kind="ExternalInput")
    w2bp = nc.dram_tensor("w2bp", [128, NK * 128], bf16, kind="ExternalInput")
    w2a = nc.dram_tensor("w2a", [128, 128], bf16, kind="ExternalInput")
    bias1 = nc.dram_tensor("bias1", [128, 128], f32, kind="ExternalInput")
    bias2 = nc.dram_tensor("bias2", [128, 128], f32, kind="ExternalInput")
    deg = nc.dram_tensor("deg", [2, PRC], f32, kind="ExternalInput")
    srcs = nc.dram_tensor("srcs", [128, 2 * NGRP * GC], i16, kind="ExternalInput")
    drel = nc.dram_tensor("drel", [128, 2 * NBLK * TB], f32, kind="ExternalInput")
    brel = nc.dram_tensor("brel", [PRC], f32, kind="ExternalInput")
    bidx = nc.dram_tensor("bidx", [128, PRC // 16], i16, kind="ExternalInput")
    rloc = nc.dram_tensor("rloc", [G], i32, kind="ExternalInput")
    iota_in = nc.dram_tensor("iota_in", [128, 128], f32, kind="ExternalInput")
    pidx = nc.dram_tensor("pidx", [128, 1], f32, kind="ExternalInput")
    fcw = nc.dram_tensor("fcw", [2, 128, 256], f32, kind="ExternalInput")
    fcb = nc.dram_tensor("fcb", [128, 2], f32, kind="ExternalInput")
    out = nc.dram_tensor("out", [G, 2], f32, kind="ExternalOutput")
    DBG = os.environ.get("KDBG", "0") == "1"
    if DBG:
        dbgY = nc.dram_tensor("dbgY", [PRC, 128], f32, kind="ExternalOutput")
        dbgZ = nc.dram_tensor("dbgZ", [PRC, 128], f32, kind="ExternalOutput")
        dbgH = nc.dram_tensor("dbgH", [PRC, 128], f32, kind="ExternalOutput")
        dbgQ = nc.dram_tensor("dbgQ", [G, 128], f32, kind="ExternalOutput")

    # ---------------- internal DRAM ----------------
    Ytl = nc.dram_tensor("Ytl", [PRC, 128], bf16)
    Ytf = nc.dram_tensor("Ytf", [NPAD, 128], bf16, addr_space="Shared")
    Ztl = nc.dram_tensor("Ztl", [PRC, 128], bf16)
    Ztf = nc.dram_tensor("Ztf", [NPAD, 128], bf16, addr_space="Shared")
    h1loc = nc.dram_tensor("h1loc", [PRC + 1, 128], f32)
    Qtab = nc.dram_tensor("Qtab", [G + 1, 128], f32)
    arl = nc.dram_tensor("arl", [128, 129], f32)
    arf = nc.dram_tensor("arf", [128, 129], f32, addr_space="Shared")
    rrl = nc.dram_tensor("rrl", [128, 128], f32)
    rrf = nc.dram_tensor("rrf", [128, 128], f32, addr_space="Shared")

    RG = [list(range(NC_))]
    NE = TB * 128  # edges (padded) per (blk, br)

    with tile.TileContext(nc) as tc:
        with tc.tile_pool(name="const", bufs=1) as cp:
            nc.gpsimd.load_library(library_config.mlp)

            # consts needed by phase G (everything else loads after G's DMAs issue)
            dga = cp.tile([128, NBLK * 2], f32)
            nc.sync.dma_start(out=dga[:], in_=deg[:].rearrange("t (b p) -> p (t b)", p=128))
            drc = cp.tile([128, NBLK * 2], f32)
            nc.vector.reciprocal(drc[:], dga[:])
            dinv = cp.tile([128, NBLK * 2], f32)  # [128, 40]: col br*NBLK+blk
            nc.scalar.activation(dinv[:], drc[:], AF.Sqrt)
            b1t = cp.tile([128, 128], f32)
            nc.sync.dma_start(out=b1t[:], in_=bias1[:])

            # local Y/Z/h1 blocks stay SBUF-resident
            ysa = cp.tile([128, NBLK, 128], bf16)
            zsa = cp.tile([128, NBLK, 128], bf16)
            ohsa = cp.tile([128, NBLK, 128], bf16)
            h1a = cp.tile([128, NBLK, 128], f32)
            ydd = cp.tile([128, NBLK, 128], f32)
            zdd = cp.tile([128, NBLK, 128], f32)
            qall = cp.tile([128, NBLK, 128], f32)

            # ---------------- phase G: U^T = W1^T x^T ; Y ----------------
            if KSTOP >= 1:
             with tc.tile_pool(name="pw", bufs=1) as pw, \
                 tc.tile_pool(name="px", bufs=3) as px, \
                 tc.tile_pool(name="pup", bufs=2, space="PSUM") as pup:
                w1all = pw.tile([128, NK * 128], bf16)
                nc.sync.dma_start(out=w1all[:], in_=w1p[:])
                for bi in range(NBLK):
                    xtc = px.tile([128, NK // 2, 256], bf16, tag="xtc")
                    nc.sync.dma_start(out=xtc[:], in_=xth[bi].rearrange("(k p) c -> p k c", p=128))
                    pu = pup.tile([128, 128], f32)
                    for k in range(NK):
                        nc.tensor.matmul(out=pu[:], lhsT=xtc[:, k // 2, (k % 2) * 128:(k % 2) * 128 + 128],
                                         rhs=w1all[:, k * 128:(k + 1) * 128], start=(k == 0), stop=(k == NK - 1))
                    yb = ysa[:, bi, :]
                    for hf in range(2):
                        dv = dinv[:, hf * NBLK + bi:hf * NBLK + bi + 1]
                        nc.vector.tensor_scalar(out=yb[:, hf * 64:hf * 64 + 64], in0=pu[:, hf * 64:hf * 64 + 64],
                                                scalar1=dv, scalar2=None, op0=OP.mult)
                        nc.vector.scalar_tensor_tensor(out=ydd[:, bi, hf * 64:hf * 64 + 64],
                                                       in0=yb[:, hf * 64:hf * 64 + 64], scalar=dv,
                                                       in1=b1t[:, hf * 64:hf * 64 + 64],
                                                       op0=OP.mult, op1=OP.add)
                    if bi == NBLK // 2 - 1:
                        nc.sync.dma_start(out=Ytl[0:PRC // 2, :].rearrange("(b p) f -> p b f", p=128),
                                          in_=ysa[:, 0:NBLK // 2, :])
                        if KSTOP >= 2:
                            nc.gpsimd.collective_compute("AllGather", OP.bypass, replica_groups=RG,
                                                         ins=[Ytl[0:PRC // 2, :]], outs=[Ytf[0:NPAD // 2, :]])
                nc.sync.dma_start(out=Ytl[PRC // 2:PRC, :].rearrange("(b p) f -> p b f", p=128),
                                  in_=ysa[:, NBLK // 2:NBLK, :])

            if KSTOP >= 2:
             nc.gpsimd.collective_compute("AllGather", OP.bypass, replica_groups=RG,
                                          ins=[Ytl[PRC // 2:PRC, :]], outs=[Ytf[NPAD // 2:NPAD, :]])

            # remaining consts (overlap G / the AG)
            iof = cp.tile([128, 128], f32)
            nc.sync.dma_start(out=iof[:], in_=iota_in[:])
            pidxt = cp.tile([128, 1], f32)
            nc.sync.dma_start(out=pidxt[:], in_=pidx[:])
            idt = cp.tile([128, 128], bf16)
            nc.vector.tensor_scalar(out=idt[:], in0=iof[:], scalar1=pidxt[:, :1],
                                    scalar2=None, op0=OP.is_equal)
            b2t = cp.tile([128, 128], f32)
            nc.sync.dma_start(out=b2t[:], in_=bias2[:])
            w2at = cp.tile([128, 128], bf16)
            nc.sync.dma_start(out=w2at[:], in_=w2a[:])
            brelt = cp.tile([128, NBLK], f32)
            nc.sync.dma_start(out=brelt[:], in_=brel[:].rearrange("(b p) -> p b", p=128))
            bidxt = cp.tile([128, PRC // 16], i16)
            nc.sync.dma_start(out=bidxt[:], in_=bidx[:])
            rloct = cp.tile([128, 1], i32)
            nc.sync.dma_start(out=rloct[:], in_=rloc[:, None])
            fcw0 = cp.tile([128, 256], f32)
            nc.sync.dma_start(out=fcw0[:], in_=fcw[0])
            fcw1 = cp.tile([128, 256], f32)
            nc.sync.dma_start(out=fcw1[:], in_=fcw[1])
            fcbt = cp.tile([128, 2], f32)
            nc.sync.dma_start(out=fcbt[:], in_=fcb[:])
            st_all = cp.tile([128, 2 * NGRP * GC], i16)
            nc.sync.dma_start(out=st_all[:], in_=srcs[:])
            dr_all = cp.tile([128, 2 * NBLK * TB], f32)
            nc.sync.dma_start(out=dr_all[:], in_=drel[:])
            zrow = cp.tile([1, 128], f32)
            nc.vector.memset(zrow[:], 0.0)
            nc.sync.dma_start(out=h1loc[PRC:PRC + 1, :], in_=zrow[:])
            nc.sync.dma_start(out=Qtab[G:G + 1, :], in_=zrow[:])

            # ---------------- phase R (overlaps AG): root rows -> Q ----------------
            if KSTOP >= 3:
             with tc.tile_pool(name="pr", bufs=1) as pr, \
                 tc.tile_pool(name="prp", bufs=1, space="PSUM") as prp:
                xrtt = pr.tile([128, NK * 128], bf16)
                nc.sync.dma_start(out=xrtt[:], in_=xrt[:])
                xrr = pr.tile([128, NK * 128], bf16)
                nc.vector.tensor_scalar_max(out=xrr[:], in0=xrtt[:], scalar1=0.0)
                w2ball = pr.tile([128, NK * 128], bf16)
                nc.sync.dma_start(out=w2ball[:], in_=w2bp[:])
                pq = prp.tile([128, 128], f32)
                for k in range(NK):
                    nc.tensor.matmul(out=pq[:], lhsT=xrr[:, k * 128:(k + 1) * 128],
                                     rhs=w2ball[:, k * 128:(k + 1) * 128],
                                     start=(k == 0), stop=(k == NK - 1))
                qsb = pr.tile([128, 128], f32)
                nc.vector.tensor_copy(qsb[:], pq[:])
                nc.sync.dma_start(out=Qtab[0:G, :], in_=qsb[:])
                # merged gather of Q[batch] for all rows (overlaps the AG)
                nc.gpsimd.dma_gather(qall[:], Qtab[:], bidxt[:], PRC, PRC, 128, single_packet=False)

            # ---------------- conv helpers ----------------
            def onehot_group(po, grp, br):
                """one big is_equal builds the one-hot for all GRP blocks of (grp, br)"""
                oh = po.tile([128, GTB, 128], bf16, tag="oh")
                d0 = (br * NBLK + grp * GRP) * TB
                nc.vector.tensor_tensor(
                    out=oh[:],
                    in0=dr_all[:, d0:d0 + GTB, None].to_broadcast([128, GTB, 128]),
                    in1=iof[:, None, :].to_broadcast([128, GTB, 128]), op=OP.is_equal)
                return oh

            def agg_block(ph, oh, V, lb, br):
                """A @ table rows for dst block (grp*GRP+lb), branch br -> psum tile [128,64]"""
                ph_ = ph.tile([128, 64], f32)
                for t in range(TB):
                    nc.tensor.matmul(out=ph_[:], lhsT=oh[:, lb * TB + t, :], rhs=V[:, lb * TB + t, br * 64:(br + 1) * 64],
                                     start=(t == 0), stop=(t == TB - 1))
                return ph_

            def gather_group(pv, table, grp, br):
                # split across the SWDGE queues: the pieces' descriptor
                # generation runs concurrently on separate Q7 contexts
                V = pv.tile([128, GTB, 128], bf16, tag="v")
                seg = br * NGRP + grp
                c0 = seg * GC
                bounds = [round(GTB * q / 4) for q in range(5)]
                for q in range(4):
                    t0, t1 = bounds[q], bounds[q + 1]
                    nc.gpsimd.dma_gather(V[:, t0:t1, :], table[:],
                                         st_all[:, c0 + t0 * 8:c0 + t1 * 8],
                                         (t1 - t0) * 128, (t1 - t0) * 128, 128,
                                         single_packet=False, queue_num=q)
                return V

            # ---------------- phase C1: conv1 -> h1, z, Zt ----------------
            if KSTOP >= 4:
             with tc.tile_pool(name="pv1", bufs=3) as pv, \
                 tc.tile_pool(name="po1", bufs=3) as po, \
                 tc.tile_pool(name="pm1", bufs=3) as pm, \
                 tc.tile_pool(name="ph1", bufs=4, space="PSUM") as ph, \
                 tc.tile_pool(name="pt1", bufs=2, space="PSUM") as pt, \
                 tc.tile_pool(name="pz1", bufs=2, space="PSUM") as pz:
                for grp in range(NGRP):
                    Vs = [gather_group(pv, Ytf, grp, br) for br in range(2)]
                    ohs_g = [onehot_group(po, grp, br) for br in range(2)]
                    for lb in range(GRP):
                        blk = grp * GRP + lb
                        h1f = h1a[:, blk, :]
                        for br in range(2):
                            ph_ = agg_block(ph, ohs_g[br], Vs[br], lb, br)
                            nc.vector.scalar_tensor_tensor(out=h1f[:, br * 64:(br + 1) * 64],
                                                           in0=ph_[:], scalar=dinv[:, br * NBLK + blk:br * NBLK + blk + 1],
                                                           in1=ydd[:, blk, br * 64:(br + 1) * 64],
                                                           op0=OP.mult, op1=OP.add)
                        hr = pm.tile([128, 128], bf16, tag="hr")
                        nc.scalar.activation(hr[:], h1f[:], AF.Relu)
                        ptp = pt.tile([128, 128], bf16)
                        nc.tensor.transpose(ptp[:], hr[:], idt[:])
                        hrT = pm.tile([128, 128], bf16, tag="hrT")
                        nc.vector.tensor_copy(hrT[:], ptp[:])
                        pz_ = pz.tile([128, 128], f32)
                        nc.tensor.matmul(out=pz_[:], lhsT=hrT[:], rhs=w2at[:], start=True, stop=True)
                        zf = pm.tile([128, 128], f32, tag="zf")
                        nc.vector.tensor_tensor(out=zf[:], in0=pz_[:], in1=qall[:, blk, :], op=OP.add)
                        ztb = zsa[:, blk, :]
                        for hf in range(2):
                            dv = dinv[:, hf * NBLK + blk:hf * NBLK + blk + 1]
                            nc.vector.tensor_scalar(out=ztb[:, hf * 64:hf * 64 + 64], in0=zf[:, hf * 64:hf * 64 + 64],
                                                    scalar1=dv, scalar2=None, op0=OP.mult)
                            nc.vector.scalar_tensor_tensor(out=zdd[:, blk, hf * 64:hf * 64 + 64],
                                                           in0=ztb[:, hf * 64:hf * 64 + 64], scalar=dv,
                                                           in1=b2t[:, hf * 64:hf * 64 + 64],
                                                           op0=OP.mult, op1=OP.add)
                        if blk == NBLK // 2 - 1:
                            nc.sync.dma_start(out=Ztl[0:PRC // 2, :].rearrange("(b p) f -> p b f", p=128),
                                              in_=zsa[:, 0:NBLK // 2, :])
                            if KSTOP >= 5:
                                nc.gpsimd.collective_compute("AllGather", OP.bypass, replica_groups=RG,
                                                             ins=[Ztl[0:PRC // 2, :]], outs=[Ztf[0:NPAD // 2, :]])
                nc.sync.dma_start(out=Ztl[PRC // 2:PRC, :].rearrange("(b p) f -> p b f", p=128),
                                  in_=zsa[:, NBLK // 2:NBLK, :])
                nc.sync.dma_start(out=h1loc[0:PRC, :].rearrange("(b p) f -> p b f", p=128), in_=h1a[:])

            if DBG and KSTOP >= 4:
             with tc.tile_pool(name="pdbg", bufs=2) as pd:
                for b in range(NBLK):
                    t1 = pd.tile([128, 128], f32, tag="t1")
                    nc.gpsimd.dma_start(out=t1[:], in_=Ytl[b * 128:(b + 1) * 128, :])
                    nc.sync.dma_start(out=dbgY[b * 128:(b + 1) * 128, :], in_=t1[:])
                    t2 = pd.tile([128, 128], f32, tag="t2")
                    nc.gpsimd.dma_start(out=t2[:], in_=Ztl[b * 128:(b + 1) * 128, :])
                    nc.sync.dma_start(out=dbgZ[b * 128:(b + 1) * 128, :], in_=t2[:])
                    t3 = pd.tile([128, 128], f32, tag="t3")
                    nc.sync.dma_start(out=t3[:], in_=h1loc[b * 128:(b + 1) * 128, :])
                    nc.sync.dma_start(out=dbgH[b * 128:(b + 1) * 128, :], in_=t3[:])
                tq = pd.tile([128, 128], f32, tag="tq")
                nc.sync.dma_start(out=tq[:], in_=Qtab[0:G, :])
                nc.sync.dma_start(out=dbgQ[:], in_=tq[:])
            if KSTOP >= 5:
             nc.gpsimd.collective_compute("AllGather", OP.bypass, replica_groups=RG,
                                          ins=[Ztl[PRC // 2:PRC, :]], outs=[Ztf[NPAD // 2:NPAD, :]])

            # segment one-hots (overlap the AG: depend only on brelt/iof)
            if KSTOP >= 6:
             for blk in range(NBLK):
                nc.vector.tensor_scalar(out=ohsa[:, blk, :], in0=iof[:],
                                        scalar1=brelt[:, blk:blk + 1], scalar2=None, op0=OP.is_equal)

            # ---------------- phase C2: conv2 -> h2 -> segment sums ----------------
            if KSTOP >= 6:
             with tc.tile_pool(name="pv2", bufs=3) as pv2, \
                 tc.tile_pool(name="po2", bufs=3) as po2, \
                 tc.tile_pool(name="pm2", bufs=3) as pm2, \
                 tc.tile_pool(name="ph2", bufs=4, space="PSUM") as ph2, \
                 tc.tile_pool(name="ps2", bufs=1, space="PSUM") as ps2:
                # root-gather of h1 + its AllReduce up front (hidden under C2)
                rg = pm2.tile([128, 128], f32, tag="rg")
                nc.gpsimd.indirect_dma_start(
                    out=rg[:], out_offset=None, in_=h1loc[:],
                    in_offset=bass.IndirectOffsetOnAxis(ap=rloct[:, :1], axis=0))
                nc.sync.dma_start(out=rrl[:], in_=rg[:])
                if KSTOP >= 7:
                    nc.gpsimd.collective_compute("AllReduce", OP.add, replica_groups=RG,
                                                 ins=[rrl[:]], outs=[rrf[:]])
                pseg = ps2.tile([128, 129], f32)
                for grp in range(NGRP):
                    Vs = [gather_group(pv2, Ztf, grp, br) for br in range(2)]
                    ohs_g = [onehot_group(po2, grp, br) for br in range(2)]
                    for lb in range(GRP):
                        blk = grp * GRP + lb
                        pay = pm2.tile([128, 129], bf16, tag="pay")
                        nc.vector.memset(pay[:, 128:129], 1.0)
                        for br in range(2):
                            ph_ = agg_block(ph2, ohs_g[br], Vs[br], lb, br)
                            hs2 = pm2.tile([128, 64], f32, tag="hs2")
                            nc.vector.scalar_tensor_tensor(out=hs2[:], in0=ph_[:],
                                                           scalar=dinv[:, br * NBLK + blk:br * NBLK + blk + 1],
                                                           in1=zdd[:, blk, br * 64:(br + 1) * 64],
                                                           op0=OP.mult, op1=OP.add)
                            nc.scalar.activation(pay[:, br * 64:(br + 1) * 64], hs2[:], AF.Relu)
                        nc.tensor.matmul(out=pseg[:], lhsT=ohsa[:, blk, :], rhs=pay[:], start=(blk == 0), stop=(blk == NBLK - 1))

                part = pm2.tile([128, 129], f32, tag="part")
                nc.vector.tensor_copy(part[:], pseg[:])
                nc.sync.dma_start(out=arl[:], in_=part[:])

            if KSTOP >= 7:
             nc.gpsimd.collective_compute("AllReduce", OP.add, replica_groups=RG,
                                          ins=[arl[:]], outs=[arf[:]])

            # ---------------- final ----------------
            if KSTOP >= 7:
             with tc.tile_pool(name="pf", bufs=1) as pf:
                Rt = pf.tile([128, 129], f32)
                nc.sync.dma_start(out=Rt[:], in_=arf[:])
                Rr = pf.tile([128, 128], f32)
                nc.sync.dma_start(out=Rr[:], in_=rrf[:])
                cnt = Rt[:, 128:129]
                c1 = pf.tile([128, 1], f32)
                nc.vector.tensor_scalar_max(out=c1[:], in0=cnt, scalar1=1.0)
                rec = pf.tile([128, 1], f32)
                nc.vector.reciprocal(rec[:], c1[:])
                ind = pf.tile([128, 1], f32)
                nc.vector.tensor_scalar_min(out=ind[:], in0=cnt, scalar1=1.0)
                hfc = pf.tile([128, 256], f32)
                nc.vector.tensor_scalar(out=hfc[:, 0:64], in0=Rt[:, 0:64], scalar1=rec[:, :1], scalar2=None, op0=OP.mult)
                nc.vector.tensor_scalar(out=hfc[:, 64:128], in0=Rr[:, 0:64], scalar1=ind[:, :1], scalar2=None, op0=OP.mult)
                nc.vector.tensor_scalar(out=hfc[:, 128:192], in0=Rt[:, 64:128], scalar1=rec[:, :1], scalar2=None, op0=OP.mult)
                nc.vector.tensor_scalar(out=hfc[:, 192:256], in0=Rr[:, 64:128], scalar1=ind[:, :1], scalar2=None, op0=OP.mult)
                lg = pf.tile([128, 2], f32)
                for j, fw in enumerate((fcw0, fcw1)):
                    tmp = pf.tile([128, 256], f32, tag=f"tmp{j}")
                    nc.vector.tensor_tensor(out=tmp[:], in0=hfc[:], in1=fw[:], op=OP.mult)
                    nc.vector.reduce_sum(lg[:, j:j + 1], tmp[:], axis=mybir.AxisListType.X)
                nc.vector.tensor_tensor(out=lg[:], in0=lg[:], in1=fcbt[:], op=OP.add)
                mx = pf.tile([128, 1], f32)
                nc.vector.reduce_max(mx[:], lg[:], axis=mybir.AxisListType.X)
                d_ = pf.tile([128, 2], f32)
                nc.vector.tensor_scalar(out=d_[:], in0=lg[:], scalar1=mx[:, :1], scalar2=None, op0=OP.subtract)
                e_ = pf.tile([128, 2], f32)
                nc.scalar.activation(e_[:], d_[:], AF.Exp)
                s_ = pf.tile([128, 1], f32)
                nc.vector.reduce_sum(s_[:], e_[:], axis=mybir.AxisListType.X)
                ls = pf.tile([128, 1], f32)
                nc.scalar.activation(ls[:], s_[:], AF.Ln)
                ov = pf.tile([128, 2], f32)
                nc.vector.tensor_scalar(out=ov[:], in0=d_[:], scalar1=ls[:, :1], scalar2=None, op0=OP.subtract)
                nc.sync.dma_start(out=out[:], in_=ov[:])

    nc.compile()
    return nc


def _prep(x, edge_index, bu_edge_index, batch, root_index,
          W1_td, b1_td, W2_td, b2_td, W1_bu, b1_bu, W2_bu, b2_bu, fc_W, fc_b):
    """Host-side: layout transforms (transpose/cast/pad) + integer index metadata."""
    x = np.asarray(x, np.float32)
    batch = np.asarray(batch).astype(np.int64)
    root_index = np.asarray(root_index).astype(np.int64)
    edges = [np.asarray(edge_index).astype(np.int64), np.asarray(bu_edge_index).astype(np.int64)]

    degs = []
    for ei in edges:
        d = np.bincount(ei[1], minlength=N).astype(np.int64) + 1
        degs.append(d)

    maxcnt = 0
    blk_edges = [[[None] * NBLK for _ in range(2)] for _ in range(NC_)]
    for br, ei in enumerate(edges):
        src, dst = ei[0], ei[1]
        c = dst // RPC
        loc = dst - c * RPC
        blk = loc // 128
        rel = loc - blk * 128
        c_s = src // RPC
        loc_s = src - c_s * RPC
        HPRC = PRC // 2
        ps = np.where(loc_s < HPRC, c_s * HPRC + loc_s,
                      NC_ * HPRC + c_s * HPRC + (loc_s - HPRC))
        key = c * NBLK + blk
        order = np.argsort(key, kind="stable")
        ks = key[order]
        bounds = np.searchsorted(ks, np.arange(NC_ * NBLK + 1))
        for c_ in range(NC_):
            for b_ in range(NBLK):
                sl = order[bounds[c_ * NBLK + b_]:bounds[c_ * NBLK + b_ + 1]]
                blk_edges[c_][br][b_] = (ps[sl], rel[sl])
                maxcnt = max(maxcnt, len(sl))
    TB = max(1, (maxcnt + 127) // 128)

    srcs_flat = np.zeros((NC_, 2, NBLK, TB * 128), np.int64)
    drel = np.full((NC_, 2, NBLK, 128, TB), -1.0, np.float32)
    for c in range(NC_):
        for br in range(2):
            for b in range(NBLK):
                s, r = blk_edges[c][br][b]
                n = len(s)
                srcs_flat[c, br, b, :n] = s
                lane, til = np.arange(n) % 128, np.arange(n) // 128
                drel[c, br, b, lane, til] = r
    # grouped gathers: concat GRP consecutive blocks per (br, grp) segment
    srcs16 = _wrap16(srcs_flat.reshape(NC_ * 2 * NGRP, GRP * TB * 128))
    srcs16 = srcs16.reshape(NC_, 2, NGRP, 128, GRP * TB * 8)
    srcs_h = np.ascontiguousarray(srcs16.transpose(0, 3, 1, 2, 4).reshape(NC_, 128, -1))
    drel_h = np.ascontiguousarray(drel.transpose(0, 3, 1, 2, 4).reshape(NC_, 128, -1))

    deg = np.full((NC_, 2, PRC), BIG, np.float32)
    for br in range(2):
        deg[:, br, :RPC] = degs[br].reshape(NC_, RPC).astype(np.float32)

    brel = np.full((NC_, PRC), -1.0, np.float32)
    brel[:, :RPC] = batch.reshape(NC_, RPC).astype(np.float32)
    bidx_flat = np.full((NC_, PRC), G, np.int64)
    bidx_flat[:, :RPC] = batch.reshape(NC_, RPC)
    bidx16 = _wrap16(bidx_flat)  # [NC_, 128, PRC//16]

    rc = root_index // RPC
    rl = root_index - rc * RPC
    rloc = np.full((NC_, G), PRC, np.int32)
    for g in range(G):
        rloc[rc[g], g] = rl[g]

    # x transposed/cast/padded per core, paired-K block layout:
    # xth[b, k2*128+p, u*128+j] = x[c*RPC + b*128 + j, (2*k2+u)*128+p]
    xts = []
    for c in range(NC_):
        xtc = np.zeros((INP, PRC), BF16)
        xtc[:IN, :RPC] = x[c * RPC:(c + 1) * RPC].T.astype(BF16)
        v = xtc.reshape(NK // 2, 2, 128, NBLK, 128)          # (k2, u, p, b, j)
        xts.append(np.ascontiguousarray(v.transpose(3, 0, 2, 1, 4)).reshape(NBLK, (NK // 2) * 128, 256))
    # root rows, K-blocked: xrt[p, k*128+g] = x[root[g], k*128+p]
    xrt = _kblocked(x[root_index].T, 128)

    # parameters (pure reshapes / replication / cast)
    w1p = _kblocked(np.hstack([np.asarray(W1_td, np.float32), np.asarray(W1_bu, np.float32)]), 128)
    w2bp = _kblocked(np.hstack([np.asarray(W2_td, np.float32)[HID:], np.asarray(W2_bu, np.float32)[HID:]]), 128)
    w2a = np.zeros((128, 128), np.float32)  # block-diag: one K=128 matmul covers both branches
    w2a[0:64, 0:64] = np.asarray(W2_td, np.float32)[:HID]
    w2a[64:128, 64:128] = np.asarray(W2_bu, np.float32)[:HID]
    w2a = w2a.astype(BF16)
    bias1 = np.broadcast_to(np.concatenate([np.asarray(b1_td, np.float32), np.asarray(b1_bu, np.float32)]), (128, 128)).copy()
    bias2 = np.broadcast_to(np.concatenate([np.asarray(b2_td, np.float32), np.asarray(b2_bu, np.float32)]), (128, 128)).copy()
    fcw = np.stack([np.broadcast_to(np.asarray(fc_W, np.float32)[:, j], (128, 256)) for j in range(2)])
    fcb = np.broadcast_to(np.asarray(fc_b, np.float32), (128, 2)).copy()
    iota_in = np.tile(np.arange(128, dtype=np.float32), (128, 1))
    pidx = np.arange(128, dtype=np.float32)[:, None].copy()

    in_maps = []
    for c in range(NC_):
        in_maps.append(dict(
            xth=xts[c], xrt=xrt,
            w1p=w1p, w2bp=w2bp, w2a=w2a, bias1=bias1, bias2=bias2,
            deg=np.ascontiguousarray(deg[c]),
            srcs=srcs_h[c], drel=drel_h[c],
            brel=np.ascontiguousarray(brel[c]),
            bidx=np.ascontiguousarray(bidx16[c]),
            rloc=np.ascontiguousarray(rloc[c]),
            iota_in=iota_in, pidx=pidx, fcw=np.ascontiguousarray(fcw), fcb=fcb,
        ))
    return TB, in_maps


def kernel(**inputs):
    from concourse.bass_utils import run_bass_kernel_spmd
    TB, in_maps = _prep(**inputs)
    if TB not in _cache:
        _cache[TB] = _build(TB)
    nc = _cache[TB]
    res = run_bass_kernel_spmd(nc, in_maps, list(range(NC_)))
    return res.results[0]["out"]


if __name__ == "__main__":
    import reference
    inputs = {k: np.asarray(v) for k, v in reference.setup_inputs().items()}
    got = kernel(**inputs)
    print(got[:4])
